# revision 2
# baseline (speedup 1.0000x reference)
"""Bass/Trainium2 kernel for DegreeOnlyFiltration (segment max + gather-divide).

Contract: kernel(**inputs) takes FULL inputs (node_deg [N] f32, sample_pos
[G+1] i32 CSR boundaries) and returns the FULL output node_deg / seg_max.

Strategy (per the sharding hint): segments are contiguous; the expected input
has uniform boundaries (sample_pos = arange(G+1) * W).  We shard node_deg by
whole segments across the 8 NeuronCores (pure data parallel, no cross-core
traffic).  On each core: view the shard as [segs_per_core, W], tile into
[128, W] SBUF tiles, reduce_max along the free axis (one segment per
partition row), reciprocal, then a per-partition-scalar multiply on the
scalar engine, and DMA the result back out.
"""

import os

import numpy as np

import concourse.bacc as bacc
import concourse.mybir as mybir
import concourse.tile as tile
from concourse.bass_utils import run_bass_kernel_spmd

N_CORES = 8
P = 128  # SBUF partitions

# Populated after each traced run (test harness reads these).
LAST_EXEC_TIME_NS = None
LAST_RESULTS = None

_NC_CACHE = {}


def _build_uniform_nc(segs_per_core: int, width: int, segs_per_tile: int):
    """SPMD program: x [segs_per_core, width] f32 -> y = x / rowmax(x).

    Each SBUF tile is [P, segs_per_tile//P * width]: partition p holds
    (segs_per_tile // P) whole segments back to back, so a single
    tensor_reduce with a 3D view yields all segment maxima in the tile.
    """
    assert segs_per_core % segs_per_tile == 0
    assert segs_per_tile % P == 0
    rows = segs_per_tile // P  # segments per partition row
    n_tiles = segs_per_core // segs_per_tile
    f32 = mybir.dt.float32

    nc = bacc.Bacc("TRN2", target_bir_lowering=False, debug=False,
                   num_devices=N_CORES)
    x = nc.dram_tensor("x", [segs_per_core, width], f32, kind="ExternalInput")
    y = nc.dram_tensor("y", [segs_per_core, width], f32, kind="ExternalOutput")

    with tile.TileContext(nc) as tc:
        with (
            tc.tile_pool(name="pin", bufs=3) as pin,
            tc.tile_pool(name="pout", bufs=3) as pout,
            tc.tile_pool(name="stats", bufs=4) as pstats,
        ):
            for t in range(n_tiles):
                s0 = t * segs_per_tile
                tin = pin.tile([P, rows * width], f32)
                nc.sync.dma_start(
                    tin[:], x[s0:s0 + segs_per_tile, :].rearrange(
                        "(p r) w -> p (r w)", p=P))
                m = pstats.tile([P, rows], f32)
                nc.vector.reduce_max(
                    m[:], tin[:].rearrange("p (r w) -> p r w", r=rows),
                    axis=mybir.AxisListType.X)
                r = pstats.tile([P, rows], f32)
                nc.vector.reciprocal(r[:], m[:])
                tout = pout.tile([P, rows * width], f32)
                for j in range(rows):
                    nc.scalar.mul(tout[:, j * width:(j + 1) * width],
                                  tin[:, j * width:(j + 1) * width],
                                  r[:, j:j + 1])
                nc.sync.dma_start(
                    y[s0:s0 + segs_per_tile, :].rearrange(
                        "(p r) w -> p (r w)", p=P), tout[:])
    nc.compile()
    return nc


def _uniform_width(sample_pos: np.ndarray, n: int):
    """Return segment width W if boundaries are uniform (pos = arange*W)."""
    if sample_pos[0] != 0 or sample_pos[-1] != n:
        return None
    diffs = np.diff(sample_pos)
    if diffs.size == 0 or np.any(diffs != diffs[0]):
        return None
    return int(diffs[0])


def _host_fallback(node_deg: np.ndarray, sample_pos: np.ndarray) -> np.ndarray:
    """Exact mirror of the reference semantics for non-uniform boundaries."""
    import jax

    with jax.default_device(jax.devices("cpu")[0]):
        import jax.numpy as jnp

        deg = jnp.asarray(node_deg)
        pos = jnp.asarray(sample_pos)
        n = deg.shape[0]
        g = pos.shape[0] - 1
        seg_ids = jnp.searchsorted(pos[1:], jnp.arange(n, dtype=pos.dtype),
                                   side="right")
        seg_max = jax.ops.segment_max(deg, seg_ids, num_segments=g)
        return np.asarray(deg / seg_max[seg_ids])


def kernel(node_deg: np.ndarray, sample_pos: np.ndarray) -> np.ndarray:
    global LAST_EXEC_TIME_NS, LAST_RESULTS

    node_deg = np.asarray(node_deg, dtype=np.float32)
    sample_pos = np.asarray(sample_pos, dtype=np.int32)
    n = node_deg.shape[0]
    g = sample_pos.shape[0] - 1

    width = _uniform_width(sample_pos, n)
    if width is None or g % N_CORES != 0 or (g // N_CORES) % P != 0:
        return _host_fallback(node_deg, sample_pos)

    segs_per_core = g // N_CORES
    # Pick segments per tile so one SBUF tile is ~2 MiB (>=1 MiB DMAs) while
    # keeping whole segments per partition row.
    rows = max(1, min(segs_per_core // P, 4096 // max(1, width)))
    segs_per_tile = P * rows
    while segs_per_core % segs_per_tile != 0:
        rows -= 1
        segs_per_tile = P * rows

    key = (segs_per_core, width, segs_per_tile)
    if key not in _NC_CACHE:
        _NC_CACHE[key] = _build_uniform_nc(*key)
    nc = _NC_CACHE[key]

    shards = node_deg.reshape(N_CORES, segs_per_core, width)
    in_maps = [{"x": shards[c]} for c in range(N_CORES)]

    trace = bool(int(os.environ.get("KERNEL_TRACE", "0")))
    try:
        res = run_bass_kernel_spmd(nc, in_maps, core_ids=list(range(N_CORES)),
                                   trace=trace)
    except Exception:
        if not trace:
            raise
        # Trace post-processing can fail in sandboxes; results still matter.
        res = run_bass_kernel_spmd(nc, in_maps, core_ids=list(range(N_CORES)),
                                   trace=False)
    LAST_EXEC_TIME_NS = res.exec_time_ns
    LAST_RESULTS = res
    out = np.concatenate([res.results[c]["y"].reshape(-1)
                          for c in range(N_CORES)])
    return out.astype(np.float32, copy=False)


# revision 3
# speedup vs baseline: 1.1465x; 1.1465x over previous
"""Bass/Trainium2 kernel for DegreeOnlyFiltration (segment max + gather-divide).

Contract: kernel(**inputs) takes FULL inputs (node_deg [N] f32, sample_pos
[G+1] i32 CSR boundaries) and returns the FULL output node_deg / seg_max.

Strategy (per the sharding hint): segments are contiguous; the expected input
has uniform boundaries (sample_pos = arange(G+1) * W).  We shard node_deg by
whole segments across the 8 NeuronCores (pure data parallel, no cross-core
traffic).  On each core: view the shard as [segs_per_core, W], tile into
[128, W] SBUF tiles, reduce_max along the free axis (one segment per
partition row), reciprocal, then a per-partition-scalar multiply on the
scalar engine, and DMA the result back out.
"""

import os

import numpy as np

import concourse.bacc as bacc
import concourse.mybir as mybir
import concourse.tile as tile
from concourse.bass_utils import run_bass_kernel_spmd

N_CORES = 8
P = 128  # SBUF partitions

# Populated after each traced run (test harness reads these).
LAST_EXEC_TIME_NS = None
LAST_RESULTS = None

_NC_CACHE = {}


def _build_uniform_nc(segs_per_core: int, width: int, segs_per_tile: int):
    """SPMD program: x [segs_per_core, width] f32 -> y = x / rowmax(x).

    Each SBUF tile is [P, segs_per_tile//P * width]: partition p holds
    (segs_per_tile // P) whole segments back to back, so a single
    tensor_reduce with a 3D view yields all segment maxima in the tile.
    """
    assert segs_per_core % segs_per_tile == 0
    assert segs_per_tile % P == 0
    rows = segs_per_tile // P  # segments per partition row
    n_tiles = segs_per_core // segs_per_tile
    f32 = mybir.dt.float32

    nc = bacc.Bacc("TRN2", target_bir_lowering=False, debug=False,
                   num_devices=N_CORES)
    x = nc.dram_tensor("x", [segs_per_core, width], f32, kind="ExternalInput")
    y = nc.dram_tensor("y", [segs_per_core, width], f32, kind="ExternalOutput")

    with tile.TileContext(nc) as tc:
        with (
            tc.tile_pool(name="pin", bufs=n_tiles) as pin,
            tc.tile_pool(name="pout", bufs=n_tiles) as pout,
            tc.tile_pool(name="stats", bufs=n_tiles) as pstats,
        ):
            # All input DMAs up front on the SP HWDGE ring: no buffer
            # recycling, no head-of-line blocking behind output DMAs.
            tins = []
            for t in range(n_tiles):
                s0 = t * segs_per_tile
                tin = pin.tile([P, rows * width], f32)
                nc.sync.dma_start(
                    tin[:], x[s0:s0 + segs_per_tile, :].rearrange(
                        "(p r) w -> p (r w)", p=P))
                tins.append(tin)
            for t in range(n_tiles):
                s0 = t * segs_per_tile
                tin = tins[t]
                m = pstats.tile([P, rows], f32)
                nc.vector.reduce_max(
                    m[:], tin[:].rearrange("p (r w) -> p r w", r=rows),
                    axis=mybir.AxisListType.X)
                r = pstats.tile([P, rows], f32)
                nc.vector.reciprocal(r[:], m[:])
                tout = pout.tile([P, rows * width], f32)
                for j in range(rows):
                    nc.scalar.mul(tout[:, j * width:(j + 1) * width],
                                  tin[:, j * width:(j + 1) * width],
                                  r[:, j:j + 1])
                # Output DMAs issue from the scalar engine -> the ACT HWDGE
                # ring, separate FIFO from the input stream.
                nc.scalar.dma_start(
                    y[s0:s0 + segs_per_tile, :].rearrange(
                        "(p r) w -> p (r w)", p=P), tout[:])
    nc.compile()
    return nc


def _uniform_width(sample_pos: np.ndarray, n: int):
    """Return segment width W if boundaries are uniform (pos = arange*W)."""
    if sample_pos[0] != 0 or sample_pos[-1] != n:
        return None
    diffs = np.diff(sample_pos)
    if diffs.size == 0 or np.any(diffs != diffs[0]):
        return None
    return int(diffs[0])


def _host_fallback(node_deg: np.ndarray, sample_pos: np.ndarray) -> np.ndarray:
    """Exact mirror of the reference semantics for non-uniform boundaries."""
    import jax

    with jax.default_device(jax.devices("cpu")[0]):
        import jax.numpy as jnp

        deg = jnp.asarray(node_deg)
        pos = jnp.asarray(sample_pos)
        n = deg.shape[0]
        g = pos.shape[0] - 1
        seg_ids = jnp.searchsorted(pos[1:], jnp.arange(n, dtype=pos.dtype),
                                   side="right")
        seg_max = jax.ops.segment_max(deg, seg_ids, num_segments=g)
        return np.asarray(deg / seg_max[seg_ids])


def kernel(node_deg: np.ndarray, sample_pos: np.ndarray) -> np.ndarray:
    global LAST_EXEC_TIME_NS, LAST_RESULTS

    node_deg = np.asarray(node_deg, dtype=np.float32)
    sample_pos = np.asarray(sample_pos, dtype=np.int32)
    n = node_deg.shape[0]
    g = sample_pos.shape[0] - 1

    width = _uniform_width(sample_pos, n)
    if width is None or g % N_CORES != 0 or (g // N_CORES) % P != 0:
        return _host_fallback(node_deg, sample_pos)

    segs_per_core = g // N_CORES
    # Pick segments per tile so one SBUF tile is ~2 MiB (>=1 MiB DMAs) while
    # keeping whole segments per partition row.
    rows = max(1, min(segs_per_core // P, 4096 // max(1, width)))
    segs_per_tile = P * rows
    while segs_per_core % segs_per_tile != 0:
        rows -= 1
        segs_per_tile = P * rows

    key = (segs_per_core, width, segs_per_tile)
    if key not in _NC_CACHE:
        _NC_CACHE[key] = _build_uniform_nc(*key)
    nc = _NC_CACHE[key]

    shards = node_deg.reshape(N_CORES, segs_per_core, width)
    in_maps = [{"x": shards[c]} for c in range(N_CORES)]

    trace = bool(int(os.environ.get("KERNEL_TRACE", "0")))
    try:
        res = run_bass_kernel_spmd(nc, in_maps, core_ids=list(range(N_CORES)),
                                   trace=trace)
    except Exception:
        if not trace:
            raise
        # Trace post-processing can fail in sandboxes; results still matter.
        res = run_bass_kernel_spmd(nc, in_maps, core_ids=list(range(N_CORES)),
                                   trace=False)
    LAST_EXEC_TIME_NS = res.exec_time_ns
    LAST_RESULTS = res
    out = np.concatenate([res.results[c]["y"].reshape(-1)
                          for c in range(N_CORES)])
    return out.astype(np.float32, copy=False)
